# revision 1
# baseline (speedup 1.0000x reference)
"""Trainium2 Bass kernel for the Tsit5 Neural-ODE problem.

Shards the batch (1024) across 8 NeuronCores (128 per core); the MLP weights
are replicated. The sequential Tsit5 scan runs fully unrolled on-device.

Restructured algebra (validated to ~1.5e-7 vs the jax reference):
  - state y kept feature-major [D=64 partitions, B=128 free] in SBUF
  - k_j is never materialized: W1 @ k_j = (W1 @ W3) @ h2_j, so all
    Runge-Kutta stage combinations accumulate directly into the next
    stage's pre-activation PSUM bank via pre-scaled weight variants
    (h*A_sj*(W1@W3))^T.  Per-stage critical path is just
    tanh -> matmul(W2) -> tanh -> matmul(W13 variant).
  - b3 contributions fold into the first tanh's per-partition bias.
  - y_{t+1} accumulates in PSUM as sum_j (h*B_j*W3) @ h2_j and one DVE
    scalar_tensor_tensor adds (Y + h*b3) + y_t.
  - the next substep's stage-1 pre-activation W1 @ y_{t+1} is carried
    recursively: W1@y_{t+1} = W1@y_t + sum_j (h*B_j*W13) @ h2_j, keeping
    the substep boundary on the same 4-hop critical path.
"""

import os

import numpy as np

import concourse.bacc as bacc
import concourse.mybir as mybir
import concourse.tile as tile
from concourse.bass import ts as _ts
from concourse.bass_utils import run_bass_kernel_spmd

f32 = mybir.dt.float32
bf16 = mybir.dt.bfloat16
ADD = mybir.AluOpType.add
TANH = mybir.ActivationFunctionType.Tanh

D, W, B, T = 64, 128, 1024, 64
N_CORES = 8
BC = B // N_CORES  # batch per core
SUBSTEPS = 2

# Tsit5 (Tsitouras 2011) tableau
A21 = 0.161
A31 = -0.008480655492356989; A32 = 0.335480655492357
A41 = 2.8971530571054935;    A42 = -6.359448489975075;  A43 = 4.3622954328695815
A51 = 5.325864828439257;     A52 = -11.748883564062828; A53 = 7.4955393428898365; A54 = -0.09249506636175525
A61 = 5.86145544294642;      A62 = -12.92096931784711;  A63 = 8.159367898576159;  A64 = -0.071584973281401; A65 = -0.028269050394068383
B1 = 0.09646076681806523; B2 = 0.01; B3 = 0.4798896504144996
B4 = 1.379008574103742;   B5 = -3.290069515436081; B6 = 2.324710524099774

_A = np.zeros((7, 7))
_A[2, 1] = A21
_A[3, 1], _A[3, 2] = A31, A32
_A[4, 1], _A[4, 2], _A[4, 3] = A41, A42, A43
_A[5, 1], _A[5, 2], _A[5, 3], _A[5, 4] = A51, A52, A53, A54
_A[6, 1], _A[6, 2], _A[6, 3], _A[6, 4], _A[6, 5] = A61, A62, A63, A64, A65
_Bv = np.array([0.0, B1, B2, B3, B4, B5, B6])

PAIRS = [(s, j) for s in range(2, 7) for j in range(1, s)]  # 15 (stage, source) pairs
PAIR_IDX = {p: i for i, p in enumerate(PAIRS)}

LAST_EXEC_NS = None
LAST_RESULTS = None
LAST_NC = None
LAST_IN_MAPS = None


def _bf16_mode():
    # "0": all fp32; "fanout": sv/sb1/wb3+hh bf16; "mm2": also W2/h1 bf16
    return os.environ.get("TSIT5_BF16", "0")


def _build(nsub):
    """Build the SPMD Bass program (identical on all cores)."""
    nt_out = nsub // SUBSTEPS + 1
    nt_full = T if os.environ.get("TSIT5_NTPAD") else nt_out
    nslot = 2 * nt_out  # y history slots, padded even for the output gather

    mode = _bf16_mode()
    fdt = bf16 if mode in ("fanout", "mm2") else f32  # fanout weights + hh
    mdt = bf16 if mode == "mm2" else f32  # W2 + h1
    NSPLIT = int(os.environ.get("TSIT5_SPLIT", "1"))
    HB = BC // NSPLIT if NSPLIT > 1 else BC  # leading half width

    nc = bacc.Bacc("TRN2")
    y0t_d = nc.declare_dram_parameter("y0t", [D, BC], f32, isOutput=False)
    w1t_d = nc.declare_dram_parameter("w1t", [D, W], f32, isOutput=False)
    w2t_d = nc.declare_dram_parameter("w2t", [W, W], mdt, isOutput=False)
    sv_d = nc.declare_dram_parameter("sv", [W, len(PAIRS) * W], fdt, isOutput=False)
    sb1_d = nc.declare_dram_parameter("sb1", [W, 6 * W], fdt, isOutput=False)
    wb3_d = nc.declare_dram_parameter("wb3", [W, 6 * D], fdt, isOutput=False)
    b1e_d = nc.declare_dram_parameter("b1e", [W, 7], f32, isOutput=False)
    b2_d = nc.declare_dram_parameter("b2v", [W, 1], f32, isOutput=False)
    cn_d = nc.declare_dram_parameter("cn", [D, 1], f32, isOutput=False)
    out_d = nc.declare_dram_parameter("out", [nt_full, D, BC], f32, isOutput=True)

    with tile.TileContext(nc) as tc:
        with (
            tc.tile_pool(name="const", bufs=1) as cpool,
            tc.tile_pool(name="state", bufs=1) as spool,
            tc.tile_pool(name="work", bufs=2) as wpool,
            tc.tile_pool(name="pp1", bufs=2, space="PSUM") as pp1,
            tc.tile_pool(name="pps", bufs=4, space="PSUM") as pps,
            tc.tile_pool(name="pph", bufs=1, space="PSUM") as pph,
            tc.tile_pool(name="ppy", bufs=1, space="PSUM") as ppy,
        ):
            w1t = cpool.tile([D, W], f32, name="w1t")
            w2t = cpool.tile([W, W], mdt, name="w2t")
            sv = cpool.tile([W, len(PAIRS) * W], fdt, name="sv")
            sb1 = cpool.tile([W, 6 * W], fdt, name="sb1")
            wb3 = cpool.tile([W, 6 * D], fdt, name="wb3")
            b1e = cpool.tile([W, 7], f32, name="b1e")
            b2v = cpool.tile([W, 1], f32, name="b2v")
            cn = cpool.tile([D, 1], f32, name="cn")
            yall = spool.tile([D, nslot * BC], f32, name="yall")

            nc.sync.dma_start(w1t[:], w1t_d[:])
            nc.sync.dma_start(w2t[:], w2t_d[:])
            nc.sync.dma_start(sv[:], sv_d[:])
            nc.sync.dma_start(sb1[:], sb1_d[:])
            nc.sync.dma_start(wb3[:], wb3_d[:])
            nc.sync.dma_start(b1e[:], b1e_d[:])
            nc.sync.dma_start(b2v[:], b2_d[:])
            nc.sync.dma_start(cn[:], cn_d[:])
            nc.sync.dma_start(yall[:, 0:BC], y0t_d[:])

            # bootstrap: P_1 for substep 0 is just W1 @ y0
            p1 = pp1.tile([W, BC], f32, tag="p1", name="p1boot")
            nc.tensor.matmul(p1, w1t[:], yall[:, 0:BC], start=True, stop=True)

            for t in range(nsub):
                ycur = yall[:, _ts(t, BC)]
                ynext = yall[:, _ts(t + 1, BC)]
                last = t == nsub - 1
                pbank = {}

                def init_p(s, ycur=ycur, pbank=pbank):
                    pb = pps.tile([W, BC], f32, tag="ps", name=f"p{s}")
                    nc.tensor.matmul(pb, w1t[:], ycur, start=True, stop=False)
                    pbank[s] = pb

                init_p(2)
                hp = pph.tile([W, BC], f32, tag="hp", name="hp")
                yacc = ppy.tile([D, BC], f32, tag="yacc", name="yacc")
                p1n = None
                hhs = {}
                for j in range(1, 7):
                    pj = p1 if j == 1 else pbank[j]
                    bias_col = (0 if t == 0 else 6) if j == 1 else j - 1
                    bias_ap = b1e[:, bias_col : bias_col + 1]
                    h1 = wpool.tile([W, BC], mdt, tag="h1", name="h1")
                    hh = wpool.tile([W, BC], fdt, tag="hh", bufs=8, name="hh")
                    # batch-halved chain ops: the first half leads the critical
                    # path; second halves trail without blocking it.
                    nc.scalar.activation(
                        h1[:, 0:HB], pj[:, 0:HB], TANH, bias=bias_ap, scale=1.0
                    )
                    nc.tensor.matmul(
                        hp[:, 0:HB], w2t[:], h1[:, 0:HB], start=True, stop=True
                    )
                    if NSPLIT > 1:
                        nc.scalar.activation(
                            h1[:, HB:BC], pj[:, HB:BC], TANH, bias=bias_ap, scale=1.0
                        )
                    if j == 1 and not last:
                        p1n = pp1.tile([W, BC], f32, tag="p1", name="p1n")
                        nc.tensor.matmul(p1n, w1t[:], ycur, start=True, stop=False)
                    if j + 2 <= 6:
                        init_p(j + 2)
                    if NSPLIT > 1:
                        nc.tensor.matmul(
                            hp[:, HB:BC], w2t[:], h1[:, HB:BC], start=True, stop=True
                        )
                    nc.scalar.activation(
                        hh[:, 0:HB], hp[:, 0:HB], TANH, bias=b2v[:, 0:1], scale=1.0
                    )
                    hhs[j] = hh
                    # older-source contributions into the next stage's bank:
                    # ready long ago, execute inside PE idle windows before the
                    # critical contribution becomes ready.
                    if j < 6:
                        for jj in range(1, j):
                            nc.tensor.matmul(
                                pbank[j + 1],
                                sv[:, _ts(PAIR_IDX[(j + 1, jj)], W)],
                                hhs[jj],
                                start=False,
                                stop=False,
                            )
                        crit_t, crit_w = pbank[j + 1], sv[:, _ts(PAIR_IDX[(j + 1, j)], W)]
                    elif not last:
                        crit_t, crit_w = p1n, sb1[:, _ts(5, W)]
                    else:
                        crit_t = None
                    if crit_t is not None:
                        nc.tensor.matmul(
                            crit_t[:, 0:HB],
                            crit_w,
                            hh[:, 0:HB],
                            start=False,
                            stop=(NSPLIT == 1),
                        )
                    if NSPLIT > 1:
                        nc.scalar.activation(
                            hh[:, HB:BC], hp[:, HB:BC], TANH, bias=b2v[:, 0:1], scale=1.0
                        )
                        if crit_t is not None:
                            nc.tensor.matmul(
                                crit_t[:, HB:BC],
                                crit_w,
                                hh[:, HB:BC],
                                start=False,
                                stop=True,
                            )
                    if not last and j < 6:
                        nc.tensor.matmul(
                            p1n, sb1[:, _ts(j - 1, W)], hh, start=False, stop=False
                        )
                    nc.tensor.matmul(
                        yacc, wb3[:, _ts(j - 1, D)], hh, start=(j == 1), stop=(j == 6)
                    )
                nc.vector.scalar_tensor_tensor(
                    ynext, yacc, cn[:, 0:1], ycur, op0=ADD, op1=ADD
                )
                p1 = p1n

            src = yall[:].rearrange("p (t two b) -> p t two b", two=2, b=BC)[
                :, :nt_out, 0, :
            ]
            nc.sync.dma_start(
                out_d[:][0:nt_out].rearrange("t d b -> d t b"), src
            )

    nc.finalize()
    return nc


def kernel(**inputs):
    global LAST_EXEC_NS, LAST_RESULTS
    ts_in = np.asarray(inputs["ts"], np.float64)
    y0 = np.asarray(inputs["y0"], np.float32)
    W1 = np.asarray(inputs["W1"], np.float64)
    b1 = np.asarray(inputs["b1"], np.float64)
    W2 = np.asarray(inputs["W2"], np.float64)
    b2 = np.asarray(inputs["b2"], np.float64)
    W3 = np.asarray(inputs["W3"], np.float64)
    b3 = np.asarray(inputs["b3"], np.float64)

    hs = np.diff(ts_in) / SUBSTEPS
    h = float(hs.mean())
    assert np.allclose(hs, h, rtol=1e-3, atol=1e-12), "kernel assumes uniform ts"

    nsub = int(os.environ.get("TSIT5_NSUB", (ts_in.shape[0] - 1) * SUBSTEPS))
    nt_out = nsub // SUBSTEPS + 1

    W13 = W1 @ W3  # [W, W]
    W1b3 = W1 @ b3  # [W]
    sigma = _A.sum(axis=1)
    sigN = _Bv.sum()

    sv_np = np.concatenate(
        [(h * _A[s, j] * W13).T for (s, j) in PAIRS], axis=1
    ).astype(np.float32)
    sb1_np = np.concatenate(
        [(h * _Bv[j] * W13).T for j in range(1, 7)], axis=1
    ).astype(np.float32)
    wb3_np = np.concatenate(
        [(h * _Bv[j] * W3).T for j in range(1, 7)], axis=1
    ).astype(np.float32)
    b1e_cols = (
        [b1]
        + [b1 + h * sigma[s] * W1b3 for s in range(2, 7)]
        + [b1 + h * sigN * W1b3]
    )
    b1e_np = np.stack(b1e_cols, axis=1).astype(np.float32)
    b2_np = b2.reshape(W, 1).astype(np.float32)
    cn_np = (h * sigN * b3).reshape(D, 1).astype(np.float32)
    w1t_np = np.ascontiguousarray(W1.T).astype(np.float32)
    w2t_np = np.ascontiguousarray(W2.T).astype(np.float32)

    nc = _build(nsub)

    import ml_dtypes

    mode = _bf16_mode()
    fcast = (lambda a: a.astype(ml_dtypes.bfloat16)) if mode in ("fanout", "mm2") else (lambda a: a)
    mcast = (lambda a: a.astype(ml_dtypes.bfloat16)) if mode == "mm2" else (lambda a: a)
    shared = {
        "w1t": w1t_np,
        "w2t": mcast(w2t_np),
        "sv": fcast(np.ascontiguousarray(sv_np)),
        "sb1": fcast(np.ascontiguousarray(sb1_np)),
        "wb3": fcast(np.ascontiguousarray(wb3_np)),
        "b1e": np.ascontiguousarray(b1e_np),
        "b2v": b2_np,
        "cn": cn_np,
    }
    in_maps = []
    for c in range(N_CORES):
        shard = y0[c * BC : (c + 1) * BC]  # [BC, D]
        m = dict(shared)
        m["y0t"] = np.ascontiguousarray(shard.T)
        in_maps.append(m)

    global LAST_NC, LAST_IN_MAPS
    LAST_NC = nc
    LAST_IN_MAPS = in_maps
    res = run_bass_kernel_spmd(nc, in_maps, list(range(N_CORES)))
    LAST_EXEC_NS = res.exec_time_ns
    LAST_RESULTS = res
    outs = [res.results[i]["out"][:nt_out] for i in range(N_CORES)]
    full = np.concatenate([o.transpose(0, 2, 1) for o in outs], axis=1)
    return np.ascontiguousarray(full.astype(np.float32))


if __name__ == "__main__":
    rng = np.random.default_rng(0)
    demo = {
        "ts": np.linspace(0.0, 1.0, T, dtype=np.float32),
        "y0": rng.standard_normal((B, D), dtype=np.float32),
        "W1": (rng.standard_normal((W, D)) / np.sqrt(D)).astype(np.float32),
        "b1": (rng.standard_normal(W) * 0.01).astype(np.float32),
        "W2": (rng.standard_normal((W, W)) / np.sqrt(W)).astype(np.float32),
        "b2": (rng.standard_normal(W) * 0.01).astype(np.float32),
        "W3": (rng.standard_normal((D, W)) / np.sqrt(W)).astype(np.float32),
        "b3": (rng.standard_normal(D) * 0.01).astype(np.float32),
    }
    out = kernel(**demo)
    print("kernel out", out.shape, out.dtype, "exec_ns:", LAST_EXEC_NS)

